# revision 1
# baseline (speedup 1.0000x reference)
"""Trainium2 Bass kernel for the Decoder (gather + shared-MLP over agents).

Math:
  assigned[b,n] = abs_actions[b, assign[b,n]]                    (gather, A=16)
  out[b,n,:]    = relu(assigned[b,n]*W1[0,:] + emb[n,:]@W1[1:,:] + b1) @ W2 + b2

Factorization used on device (per core, N sharded 8 ways -> NC=1250):
  embW[n,h]   = emb[n,:]@W1[1:,h] (+ b1 folded into emb on host when nonzero)
  relu(x + e) = max(x, -e) + e
  out[b,n,o]  = sum_h max(s[b,n]*W1_0[h], -embW[n,h]) * W2b[h,o]
                + corr[n,o],   corr = embW@W2 + b2  (batch-independent)

Device pipeline per batch b:
  GPSIMD : broadcast s[b,:] across the 128 partitions (h on partitions)
  DVE    : scalar_tensor_tensor  t = (s_bcast * W1_0[h]) max negEmbW  (bf16 2x)
  PE     : out = W2.T @ t, col-packed 4x via tile_position, + corr (K=2 mm)
  ACT    : drain PSUM -> SBUF staging
  DMA    : staging -> DRAM (one DMA per batch, issued from the GP queue)

The gather is a one-hot matmul: onehot[(b,a), n] = (assign[b,n]==a) built by
one DVE compare per 128-row tile, contracted with a block-diagonal
abs_actions matrix on the PE.
"""

import sys

sys.path.insert(0, "/opt/trn_rl_repo")

import numpy as np
import ml_dtypes

import concourse.bass as bass
import concourse.tile as tile
import concourse.mybir as mybir
from concourse import bacc
from concourse.bass_utils import run_bass_kernel_spmd

BF16 = ml_dtypes.bfloat16

B, A, N, E, H, OUT = 32, 16, 10000, 256, 256, 2
NCORES = 8
NC = N // NCORES  # 1250 real columns per core
NP = 1280  # padded to 4 * 320 for regular chunking
P = 128

CH = [0, 512, 1024, NP]  # chunks for K=256 matmuls staged through PSUM
CG = [0, 320, 640, 960, NP]  # column groups for the col-packed consume

_CACHE = {}


def build_program():
    """Build the Bass/Tile program once (shared by all 8 cores, SPMD)."""
    nc = bacc.Bacc("TRN2", target_bir_lowering=False, debug=False)
    f32 = mybir.dt.float32
    bf16 = mybir.dt.bfloat16

    d_embT = nc.dram_tensor("embT", (2, P, NP), f32, kind="ExternalInput").ap()
    d_arep = nc.dram_tensor("assign_rep", (4, P, NP), bf16, kind="ExternalInput").ap()
    d_absf = nc.dram_tensor("absflat", (4, P, B), bf16, kind="ExternalInput").ap()
    d_iota = nc.dram_tensor("iota16", (P, 1), f32, kind="ExternalInput").ap()
    d_w1e = nc.dram_tensor("w1e", (2, P, H), f32, kind="ExternalInput").ap()
    d_w10 = nc.dram_tensor("w1_0c", (P, 2), f32, kind="ExternalInput").ap()
    d_w2b = nc.dram_tensor("w2b", (2, P, 32), bf16, kind="ExternalInput").ap()
    d_w2f = nc.dram_tensor("w2f", (2, P, OUT), f32, kind="ExternalInput").ap()
    d_eye2 = nc.dram_tensor("eye2", (OUT, 32), f32, kind="ExternalInput").ap()
    d_b2 = nc.dram_tensor("b2c", (OUT, 1), f32, kind="ExternalInput").ap()
    d_out = nc.dram_tensor("out", (B, P, 320), f32, kind="ExternalOutput").ap()

    mm = mybir.AluOpType

    with tile.TileContext(nc) as tc:
        with (
            tc.tile_pool(name="const", bufs=1) as cpool,
            tc.tile_pool(name="work", bufs=1) as wpool,
            tc.tile_pool(name="sbc", bufs=3) as sbcp,
            tc.tile_pool(name="ostg", bufs=3) as ostgp,
            tc.tile_pool(name="tt", bufs=2) as ttp,
            tc.tile_pool(name="ps_pro", bufs=2, space="PSUM") as pspro,
            tc.tile_pool(name="ps_out", bufs=2, space="PSUM") as psout,
        ):
            # ---- load constants / inputs ----
            embT = cpool.tile([P, 2, NP], f32)
            arep = cpool.tile([P, 4, NP], bf16)
            absf = cpool.tile([P, 4, B], bf16)
            iota = cpool.tile([P, 1], f32)
            w1e = cpool.tile([P, 2, H], f32)
            w10 = cpool.tile([P, 2], f32)
            w2b = cpool.tile([P, 2, 32], bf16)
            w2f = cpool.tile([P, 2, OUT], f32)
            eye2 = cpool.tile([OUT, 32], f32)
            b2c = cpool.tile([OUT, 1], f32)

            for k in range(2):
                nc.sync.dma_start(embT[:, k, :], d_embT[k])
                nc.sync.dma_start(w1e[:, k, :], d_w1e[k])
                nc.sync.dma_start(w2b[:, k, :], d_w2b[k])
                nc.sync.dma_start(w2f[:, k, :], d_w2f[k])
            for t in range(4):
                nc.sync.dma_start(arep[:, t, :], d_arep[t])
                nc.sync.dma_start(absf[:, t, :], d_absf[t])
            nc.sync.dma_start(iota[:], d_iota[:])
            nc.sync.dma_start(w10[:], d_w10[:])
            nc.sync.dma_start(eye2[:], d_eye2[:])
            nc.sync.dma_start(b2c[:], d_b2[:])

            # ---- working tensors ----
            onehot = wpool.tile([P, 4, NP], bf16)
            s_all = wpool.tile([B, NP], bf16)
            s_flat = wpool.tile([1, B, NP], bf16)
            embW = wpool.tile([P, 2, NP], f32)
            negEW = wpool.tile([P, 2, NP], bf16)
            corr = wpool.tile([OUT, NP], f32)

            # ---- one-hot of assignments: onehot[(b,a), n] = (assign[b,n]==a)
            for t in range(4):
                nc.vector.tensor_scalar(
                    onehot[:, t, :], arep[:, t, :], iota[:, 0:1], None, mm.is_equal
                )

            # ---- gather s[b,n] = abs_actions[b, assign[b,n]] via matmul ----
            for ci in range(3):
                w = CH[ci + 1] - CH[ci]
                ps = pspro.tile([B, 512], f32, tag="ps_s")
                for t in range(4):
                    nc.tensor.matmul(
                        ps[:, :w],
                        absf[:, t, :],
                        onehot[:, t, CH[ci] : CH[ci + 1]],
                        start=(t == 0),
                        stop=(t == 3),
                    )
                nc.scalar.copy(s_all[:, CH[ci] : CH[ci + 1]], ps[:, :w])
            # relayout so every batch row sits at partition 0 (GPSIMD ISA ops
            # require partition-0-aligned operands); one trivial DMA per row —
            # a single 32->1-partition gather DMA corrupts on this runtime
            for b in range(B):
                nc.scalar.dma_start(s_flat[0:1, b, :], s_all[b : b + 1, :])

            # ---- embW = emb @ W1[1:]  (transposed: h on partitions) ----
            for t in range(2):
                for ci in range(3):
                    w = CH[ci + 1] - CH[ci]
                    ps = pspro.tile([P, 512], f32, tag="ps_e")
                    for k in range(2):
                        nc.tensor.matmul(
                            ps[:, :w],
                            w1e[:, k, t * P : (t + 1) * P],
                            embT[:, k, CH[ci] : CH[ci + 1]],
                            start=(k == 0),
                            stop=(k == 1),
                        )
                    nc.scalar.mul(negEW[:, t, CH[ci] : CH[ci + 1]], ps[:, :w], -1.0)
                    nc.scalar.copy(embW[:, t, CH[ci] : CH[ci + 1]], ps[:, :w])

            # ---- corr = embW @ W2 + b2 (fp32, exact) ----
            for ci in range(3):
                w = CH[ci + 1] - CH[ci]
                ps = pspro.tile([OUT, 512], f32, tag="ps_c")
                for k in range(2):
                    nc.tensor.matmul(
                        ps[:, :w],
                        w2f[:, k, :],
                        embW[:, k, CH[ci] : CH[ci + 1]],
                        start=(k == 0),
                        stop=(k == 1),
                    )
                nc.scalar.activation(
                    corr[:, CH[ci] : CH[ci + 1]],
                    ps[:, :w],
                    mybir.ActivationFunctionType.Identity,
                    bias=b2c[:, 0:1],
                    scale=1.0,
                )

            # ---- main loop over batches ----
            for b in range(B):
                sbc = sbcp.tile([P, NP], bf16, tag="sbc")
                nc.gpsimd.partition_broadcast(sbc[:], s_flat[0:1, b, :])

                ts = []
                for t in range(2):
                    tt = ttp.tile([P, NP], bf16, tag=f"t{t}")
                    nc.vector.scalar_tensor_tensor(
                        tt[:],
                        sbc[:],
                        w10[:, t : t + 1],
                        negEW[:, t, :],
                        mm.mult,
                        mm.max,
                    )
                    ts.append(tt)

                pso = psout.tile([P, 320], f32, tag="pso")
                for j in range(4):
                    nc.tensor.matmul(
                        pso[32 * j : 32 * j + 32, :],
                        eye2[:, :],
                        corr[:, CG[j] : CG[j + 1]],
                        start=True,
                        stop=False,
                        tile_position=(0, 32 * j),
                    )
                    for k in range(2):
                        nc.tensor.matmul(
                            pso[32 * j : 32 * j + 32, :],
                            w2b[:, k, :],
                            ts[k][:, CG[j] : CG[j + 1]],
                            start=False,
                            stop=(k == 1),
                            tile_position=(0, 32 * j),
                        )

                ostg = ostgp.tile([P, 320], f32, tag="ostg")
                nc.scalar.copy(ostg[:], pso[:])
                nc.sync.dma_start(d_out[b], ostg[:])

    nc.compile()
    return nc


def prep_inputs(abs_actions, assignments, emb_padded):
    """Per-core input dicts. emb_padded: (N, E) fp32 with b1 already folded."""
    in_maps = []
    for c in range(NCORES):
        sl = slice(c * NC, (c + 1) * NC)
        embT = np.zeros((E, NP), np.float32)
        embT[:, :NC] = emb_padded[sl].T
        a_sl = np.zeros((B, NP), np.int32)
        a_sl[:, :NC] = assignments[:, sl]
        arep = np.ascontiguousarray(
            a_sl[np.arange(B * A) // A].reshape(4, P, NP)
        ).astype(BF16)
        in_maps.append(
            {
                "embT": np.ascontiguousarray(embT.reshape(2, P, NP)),
                "assign_rep": arep,
                "absflat": _CACHE["absflat"],
                "iota16": _CACHE["iota16"],
                "w1e": _CACHE["w1e"],
                "w1_0c": _CACHE["w1_0c"],
                "w2b": _CACHE["w2b"],
                "w2f": _CACHE["w2f"],
                "eye2": _CACHE["eye2"],
                "b2c": _CACHE["b2c"],
            }
        )
    return in_maps


def kernel(abs_actions, abstract_agent_assignments, emb, W1, b1, W2, b2):
    abs_actions = np.asarray(abs_actions, np.float32)
    assign = np.asarray(abstract_agent_assignments).astype(np.int32)
    emb = np.asarray(emb, np.float32)
    W1 = np.asarray(W1, np.float32)
    b1 = np.asarray(b1, np.float32)
    W2 = np.asarray(W2, np.float32)
    b2 = np.asarray(b2, np.float32)

    # Fold b1 into emb: (emb + 1 v^T) @ W1[1:] = emb@W1[1:] + 1 b1^T when
    # W1[1:].T v = b1.  Exact for full-rank square W1[1:]; b1 == 0 here anyway.
    if np.any(b1 != 0):
        v = np.linalg.lstsq(W1[1:].T, b1, rcond=None)[0]
        if not np.allclose(W1[1:].T @ v, b1, atol=1e-5):
            raise ValueError("cannot fold nonzero b1 exactly")
        emb = emb + v[None, :]

    _build_consts(abs_actions, W1, W2, b2)

    if "nc" not in _CACHE:
        _CACHE["nc"] = build_program()
    nc = _CACHE["nc"]

    in_maps = prep_inputs(abs_actions, assign, emb)
    res = run_bass_kernel_spmd(nc, in_maps, list(range(NCORES))).results
    outs = np.stack([np.asarray(res[c]["out"]) for c in range(NCORES)])
    # outs: (8, B, 128, 320); row 32j+o, col nn -> out[b, 320j+nn, o]
    outs = outs.reshape(NCORES, B, 4, 32, 320)[:, :, :, :OUT, :]  # (8,B,4,2,320)
    outs = outs.transpose(1, 0, 2, 4, 3).reshape(B, NCORES, NP, OUT)  # (B,8,1280,2)
    return np.ascontiguousarray(outs[:, :, :NC, :].reshape(B, N, OUT))


def _build_consts(abs_actions, W1, W2, b2):
    absflat = np.zeros((B * A, B), np.float32)
    absflat[np.arange(B * A), np.arange(B * A) // A] = abs_actions.reshape(-1)
    _CACHE["absflat"] = np.ascontiguousarray(absflat.reshape(4, P, B)).astype(BF16)
    _CACHE["iota16"] = (np.arange(P, dtype=np.float32) % A).reshape(P, 1)
    _CACHE["w1e"] = np.ascontiguousarray(W1[1:, :].reshape(2, P, H))
    _CACHE["w1_0c"] = np.ascontiguousarray(W1[0, :].reshape(2, P).T)
    w2f = np.ascontiguousarray(W2.reshape(2, P, OUT))
    _CACHE["w2f"] = w2f
    w2pad = np.zeros((2, P, 32), np.float32)
    w2pad[:, :, :OUT] = w2f
    _CACHE["w2b"] = w2pad.astype(BF16)
    eye2 = np.zeros((OUT, 32), np.float32)
    eye2[:, :OUT] = np.eye(OUT)
    _CACHE["eye2"] = eye2
    _CACHE["b2c"] = np.ascontiguousarray(b2.reshape(OUT, 1))

